# revision 36
# baseline (speedup 1.0000x reference)
"""Trainium2 Bass kernel for fused cross+self attention (nn_Attention_3539053052516).

v3 strategy (8 NeuronCores, head-parallel, all-bf16 matmuls):
  - 16 heads -> 2 heads per core: per-core q/k/v projections, attention over
    4096 keys, partial output projection over the core's 128 o-channels;
    host sums 8 bf16 partials + bias.
  - Scores: [keys, q] with kT stationary (contraction hd=64, two heads packed
    via tile_position row halves).
  - attn@V runs in [q, d] form: stationary p-block [128 keys, 128 q], moving
    v [128 keys, 65] (ones column 64 = softmax denominator) -> each matmul is
    charged only 65 columns; o accumulates per q-subtile with per-partition
    denominators, so normalization is a cheap per-partition scale.
  - exp splits between Scalar (true Exp -> bf16) and Vector (Schraudolph
    bit-exp: round(a*s + b) -> uint16, bitcast bf16). No max-subtraction:
    a fixed bias -7 keeps everything in bf16 range (max exp arg ~17.7).
  - RMS norm weights fold into rotary coeffs (q/kx) and wy rows (ky); rstd
    folds into the rotary prescale (q/kx) and a per-token scale on ky.
  - Stats: squares on gpsimd, reduce on DVE, sqrt on Act, recip on DVE.
  - Natural->T transposes on PE (bf16), evacuated as uint16 copies (2x DVE).
"""

import numpy as np
import ml_dtypes

import concourse.bass as bass
import concourse.tile as tile
from concourse import bacc, mybir
from concourse.masks import make_identity
from concourse.bass_utils import run_bass_kernel_spmd

F32 = mybir.dt.float32
BF16 = mybir.dt.bfloat16
U16 = mybir.dt.uint16
FP16 = mybir.dt.float16
AF = mybir.ActivationFunctionType
BF = ml_dtypes.bfloat16

H = 16
HD = 64
C = 1024
NCORES = 8
HPC = H // NCORES  # 2 heads per core
EPS = 1e-6
SCALE = HD ** -0.5          # 0.125
EXP_BIAS = -7.0             # p = exp(s*SCALE + EXP_BIAS); cancels in softmax
LOG2E = 1.4426950408889634
BITEXP_A = SCALE * LOG2E * 128.0                      # bf16 Schraudolph
BITEXP_B = 128.0 * (EXP_BIAS * LOG2E + 127.0) - 5.54  # mantissa interp corr
ACT_EXP_NUM = 1             # Act handles ACT_EXP_NUM of every ACT_EXP_DEN chunks
ACT_EXP_DEN = 2
ACT_BITEXP = True           # Act uses the same bit-exp (Copy) as DVE
WARMUP_MM = 24
DEBUG = False


def build_nc(n_tok=2048, m_tok=2048, num_devices=NCORES):
    TT = n_tok // 128        # 16 token tiles per side
    KC = (n_tok + m_tok) // 128   # 32 key chunks of 128
    QB = n_tok // 512        # 4 q blocks of 512
    nc = bacc.Bacc("TRN2", target_bir_lowering=False, debug=False,
                   num_devices=num_devices)

    xT = nc.dram_tensor("xT", [128, 8 * n_tok], BF16, kind="ExternalInput").ap()
    yT = nc.dram_tensor("yT", [128, 8 * m_tok], BF16, kind="ExternalInput").ap()
    wx = nc.dram_tensor("wx", [128, 8 * 384], BF16, kind="ExternalInput").ap()
    wy = nc.dram_tensor("wy", [128, 8 * 256], BF16, kind="ExternalInput").ap()
    cq = nc.dram_tensor("cq", [128, 4 * TT * 32], BF16, kind="ExternalInput").ap()
    ck = nc.dram_tensor("ck", [128, 4 * TT * 32], BF16, kind="ExternalInput").ap()
    wp = nc.dram_tensor("wp", [128, C], BF16, kind="ExternalInput").ap()
    out_d = nc.dram_tensor("out", [n_tok, C], BF16, kind="ExternalOutput").ap()
    dbg = {}
    if DEBUG:
        for nm, shape, dt in [
                ("d_qkx", [128, TT * 256], FP16), ("d_rstdx", [128, TT * 4], F32),
                ("d_qT", [128, n_tok], FP16), ("d_kT", [128, n_tok + m_tok], FP16),
                ("d_vaug", [128, KC * 2 * 65], BF16),
                ("d_pt00", [128, 2 * 512], BF16), ("d_ops00", [128, 4 * 65], F32),
                ("d_oT", [128, n_tok], BF16)]:
            dbg[nm] = nc.dram_tensor(nm, shape, dt, kind="ExternalOutput").ap()

    with tile.TileContext(nc) as tc:
        _emit(tc, nc, locals())
    nc.compile()
    return nc


def _emit(tc, nc, g):
    n_tok, m_tok = g["n_tok"], g["m_tok"]
    TT, KC, QB = g["TT"], g["KC"], g["QB"]
    xT_d, yT_d, wx_d, wy_d = g["xT"], g["yT"], g["wx"], g["wy"]
    cq_d, ck_d, wp_d, out_d = g["cq"], g["ck"], g["wp"], g["out_d"]
    A = mybir.AluOpType

    const = tc.alloc_tile_pool(name="const", bufs=1)
    data = tc.alloc_tile_pool(name="data", bufs=1)
    work = tc.alloc_tile_pool(name="work", bufs=3)

    # ---------------- constants ----------------
    identb = const.tile([128, 128], BF16)
    make_identity(nc, identb[:])
    identh = const.tile([128, 128], FP16)
    make_identity(nc, identh[:])
    ebias = const.tile([128, 1], F32)
    nc.gpsimd.memset(ebias[:], EXP_BIAS)
    eps_t = const.tile([128, 1], F32)
    nc.gpsimd.memset(eps_t[:], EPS)

    # ---------------- input tiles ----------------
    wx_t = const.tile([128, 8, 384], BF16)
    wy_t = const.tile([128, 8, 256], BF16)
    cq_t = const.tile([128, 4, TT, 32], BF16)
    ck_t = const.tile([128, 4, TT, 32], BF16)
    wp_t = const.tile([128, C], BF16)
    xt = data.tile([128, 8, n_tok], BF16)    # xT chunks [C-chunk, tok]
    yt = data.tile([128, 8, m_tok], BF16)

    def flat(t):
        ap = t[:]
        return ap.rearrange(
            {3: "p a b -> p (a b)", 4: "p a b c -> p (a b c)"}[len(ap.shape)])

    nc.sync.dma_start(flat(wx_t), wx_d[:])
    xv = xT_d[:].rearrange("p (c t) -> p c t", c=8)
    for j in range(8):
        nc.sync.dma_start(xt[:, j:j + 1, :], xv[:, j:j + 1, :])
    nc.sync.dma_start(flat(wy_t), wy_d[:])
    yv = yT_d[:].rearrange("p (c t) -> p c t", c=8)
    for j in range(8):
        nc.sync.dma_start(yt[:, j:j + 1, :], yv[:, j:j + 1, :])
    nc.sync.dma_start(flat(cq_t), cq_d[:])
    nc.sync.dma_start(flat(ck_t), ck_d[:])
    nc.sync.dma_start(wp_t[:], wp_d[:])

    # ---------------- natural-layout buffers ----------------
    # channel order per tensor: [h0: te32 to32 | h1: te32 to32] (q, kx, ky)
    qkx = data.tile([128, TT, 256], FP16)       # raw q|kx
    kyr = data.tile([128, TT, 128], FP16)       # raw ky
    qn = data.tile([128, TT, 128], FP16)        # rotated+normed q [h: oe oo]
    kxn = data.tile([128, TT, 128], FP16)
    kyn = data.tile([128, TT, 128], FP16)
    vaug = data.tile([128, KC, 2, 65], BF16)    # [p, kc, head, 65]; col 64 = 1
    vaug_v = vaug[:]
    nc.gpsimd.memset(
        vaug_v.rearrange("p a b c -> p (a b) c")[:, :, 64:65], 1.0)

    ssx = data.tile([128, TT, 4], F32)   # sumsq: q-h0 q-h1 kx-h0 kx-h1
    rstdx = data.tile([128, TT, 4], F32)
    ssy = data.tile([128, TT, 2], F32)
    rstdy = data.tile([128, TT, 2], F32)
    rsx_v, rsy_v = rstdx[:], rstdy[:]

    qT = data.tile([128, n_tok], FP16)          # [2h x 64d, tok]
    kT = data.tile([128, n_tok + m_tok], FP16)
    oT = data.tile([128, n_tok], BF16)

    # ---------------- PE warmup (spin up P-state during input DMA) --------
    psW = tc.alloc_tile_pool(name="psW", bufs=2, space="PSUM")
    for i in range(WARMUP_MM):
        wmt = psW.tile([128, 128], BF16, tag="wm")
        nc.tensor.transpose(wmt[:], identb[:], identb[:])
    psW.release()

    # ================= phases 1-3: projections, stats, rotary, transposes ==
    psA = tc.alloc_tile_pool(name="psA", bufs=1, space="PSUM")
    psB = tc.alloc_tile_pool(name="psB", bufs=2, space="PSUM")

    sqs = work.tile([128, TT, 256], BF16, tag="sqs", bufs=1)

    def stats(side, nat, ss, rstd, width):
        # squares on gpsimd, reduce on DVE (i-blocks), add halves, sqrt, recip
        # one batched pass over all TT tiles
        src = nat[:]
        sq = sqs[:, :, 0:width]
        nc.gpsimd.tensor_mul(sq, src, src)
        nr = width // 32
        part = work.tile([128, TT, nr], F32, tag=f"hh{side}", bufs=1)
        nc.vector.reduce_sum(part[:], sq.rearrange("p t (r i) -> p t r i", i=32),
                             axis=mybir.AxisListType.X)
        pv = part[:].rearrange("p t (g half) -> p t g half", half=2)
        nc.vector.tensor_add(ss[:], pv[:, :, :, 0], pv[:, :, :, 1])
        rms = work.tile([128, TT, nr // 2], F32, tag=f"rms{side}", bufs=1)
        nc.scalar.activation(rms[:], ss[:], AF.Sqrt, scale=1.0 / HD, bias=eps_t[:])
        with nc.allow_low_precision(reason="rstd bf16"):
            nc.vector.reciprocal(rstd[:], rms[:])

    qkx_v, kyr_v = qkx[:], kyr[:]
    cqv, ckv = cq_t[:], ck_t[:]
    qn_v, kxn_v, kyn_v = qn[:], kxn[:], kyn[:]

    def rot(base, coeff, dst):
        # in [h: te32 to32] at base+64h; out [h: oe32 oo32]; rstd prescale.
        # fully batched across all TT tiles per (head, half).
        for h in range(HPC):
            g = h if base == 0 else 2 + h
            rsb = rsx_v[:, :, g:g + 1].broadcast_to((128, TT, 32))
            te = qkx_v[:, :, base + 64 * h: base + 64 * h + 32]
            to = qkx_v[:, :, base + 64 * h + 32: base + 64 * h + 64]
            tep = work.tile([128, TT, 32], F32, tag="tep", bufs=2)
            top = work.tile([128, TT, 32], F32, tag="top", bufs=2)
            nc.vector.tensor_mul(tep[:], te, rsb)
            nc.vector.tensor_mul(top[:], to, rsb)
            m1 = work.tile([128, TT, 32], F32, tag="m1", bufs=2)
            m2 = work.tile([128, TT, 32], F32, tag="m2", bufs=2)
            nc.vector.tensor_mul(m1[:], tep[:], coeff[:, 0, :, :])
            nc.vector.tensor_mul(m2[:], top[:], coeff[:, 1, :, :])
            with nc.allow_low_precision(reason="fp16 qk"):
                nc.vector.tensor_sub(dst[:, :, 64 * h:64 * h + 32], m1[:], m2[:])
            nc.vector.tensor_mul(m1[:], tep[:], coeff[:, 2, :, :])
            nc.vector.tensor_mul(m2[:], top[:], coeff[:, 3, :, :])
            with nc.allow_low_precision(reason="fp16 qk"):
                nc.vector.tensor_add(dst[:, :, 64 * h + 32:64 * h + 64],
                                     m1[:], m2[:])

    xt_v, yt_v, wxv, wyv = xt[:], yt[:], wx_t[:], wy_t[:]
    # ---- x projection (chunk-outer within 4-tile blocks: starts on the
    # first C-chunk DMA instead of waiting for all of x) ----
    for tb in range(0, TT, 4):
        pss = [psA.tile([128, 384], F32, tag=f"pj{t % 4}", name=f"pjx{t}")
               for t in range(tb, tb + 4)]
        for j in range(8):
            for ti, t in enumerate(range(tb, tb + 4)):
                nc.tensor.matmul(pss[ti][:], xt_v[:, j, t * 128:(t + 1) * 128],
                                 wxv[:, j, :], start=(j == 0), stop=(j == 7))
        for ti, t in enumerate(range(tb, tb + 4)):
            with nc.allow_low_precision(reason="fp16 qk"):
                nc.scalar.copy(qkx_v[:, t, :], pss[ti][:, 0:256])
            with nc.allow_low_precision(reason="bf16 v"):
                nc.scalar.copy(
                    vaug_v[:, t, :, 0:64],
                    pss[ti][:, 256:384].rearrange("p (h c) -> p h c", h=2))
    # ---- y projection ----
    for tb in range(0, TT, 4):
        pss = [psA.tile([128, 256], F32, tag=f"pj{t % 4}", name=f"pjy{t}")
               for t in range(tb, tb + 4)]
        for j in range(8):
            for ti, t in enumerate(range(tb, tb + 4)):
                nc.tensor.matmul(pss[ti][:], yt_v[:, j, t * 128:(t + 1) * 128],
                                 wyv[:, j, :], start=(j == 0), stop=(j == 7))
        for ti, t in enumerate(range(tb, tb + 4)):
            with nc.allow_low_precision(reason="fp16 k"):
                nc.scalar.copy(kyr_v[:, t, :], pss[ti][:, 0:128])
            with nc.allow_low_precision(reason="bf16 v"):
                nc.scalar.copy(
                    vaug_v[:, TT + t, :, 0:64],
                    pss[ti][:, 128:256].rearrange("p (h c) -> p h c", h=2))

    # ---- stats + rotary + ky norm (fully batched) ----
    stats("x", qkx, ssx, rstdx, 256)
    rot(0, cqv, qn_v)
    rot(128, ckv, kxn_v)
    stats("y", kyr, ssy, rstdy, 128)
    for h in range(HPC):
        rsb = rsy_v[:, :, h:h + 1].broadcast_to((128, TT, 64))
        with nc.allow_low_precision(reason="fp16 k"):
            nc.vector.tensor_mul(kyn_v[:, :, 64 * h:64 * h + 64],
                                 kyr_v[:, :, 64 * h:64 * h + 64], rsb)

    # ---- transposes natural -> T (batch 2 tiles per psum for wide evacs) --
    def tpose(src_v, t0, nt, dst, base_t):
        pb = psB.tile([128, 2, 128], FP16, tag="tr")
        pbv = pb[:]
        for i in range(nt):
            nc.tensor.transpose(pbv[:, i, :], src_v[:, t0 + i, :], identh[:])
        nc.vector.tensor_copy(
            dst[:, base_t + t0 * 128: base_t + (t0 + nt) * 128].bitcast(U16),
            pbv[:, 0:nt, :].bitcast(U16))

    qT_v, kT_v = qT[:], kT[:]
    for t0 in range(0, TT, 2):
        tpose(qn_v, t0, 2, qT_v, 0)
    for t0 in range(0, TT, 2):
        tpose(kxn_v, t0, 2, kT_v, 0)
    for t0 in range(0, TT, 2):
        tpose(kyn_v, t0, 2, kT_v, n_tok)

    dbg = g["dbg"]

    def dump(nm, src_ap, psum=False):
        shp = list(src_ap.shape)
        if psum:
            tmp = work.tile(shp, F32, tag=f"dbg{nm}", bufs=1, name=f"dbg{nm}")
            nc.vector.tensor_copy(tmp[:], src_ap)
            src_ap = tmp[:]
        ap = src_ap
        if len(shp) > 2:
            ap = ap.rearrange({3: "p a b -> p (a b)",
                               4: "p a b c -> p (a b c)"}[len(shp)])
        nc.sync.dma_start(dbg[nm][:], ap)

    if DEBUG:
        dump("d_qkx", qkx_v)
        dump("d_rstdx", rsx_v)
        dump("d_qT", qT_v)
        dump("d_kT", kT_v)
        dump("d_vaug", vaug_v)

    psB.release()
    psA.release()

    # ================= phase 4: attention + output projection =============
    psC = tc.alloc_tile_pool(name="psC", bufs=1, space="PSUM")
    oT_v = oT[:]
    expi = [0]
    pend_po = []

    poi = [0]

    def emit_po(qb, ts, half):
        po = psC.tile([128, 512], F32, tag="po", bufs=2, name="po")
        nc.tensor.matmul(po[:],
                         oT_v[:, qb * 512 + ts * 128: qb * 512 + (ts + 1) * 128],
                         wp_t[:, half * 512:(half + 1) * 512],
                         start=True, stop=True)
        ob = work.tile([128, 512], BF16, tag="ob", bufs=3, name="ob")
        with nc.allow_low_precision(reason="bf16 partial out"):
            if poi[0] % 2 == 0:
                nc.scalar.copy(ob[:], po[:])
            else:
                nc.vector.tensor_copy(ob[:], po[:])
        poi[0] += 1
        nc.sync.dma_start(
            out_d[qb * 512 + ts * 128: qb * 512 + (ts + 1) * 128,
                  half * 512:(half + 1) * 512], ob[:])

    def emit_exp(pt, scv):
        i0 = expi[0]
        expi[0] += 1
        if (i0 * ACT_EXP_NUM) // ACT_EXP_DEN != \
                ((i0 + 1) * ACT_EXP_NUM) // ACT_EXP_DEN:
            if ACT_BITEXP:
                with nc.allow_low_precision(reason="bit-exp"):
                    nc.scalar.activation(pt[:].bitcast(U16), scv, AF.Copy,
                                         bias=BITEXP_B, scale=BITEXP_A)
            else:
                nc.scalar.activation(pt[:], scv, AF.Exp,
                                     scale=SCALE, bias=ebias[:])
        else:
            with nc.allow_low_precision(reason="bit-exp"):
                nc.vector.tensor_scalar(pt[:].bitcast(U16), scv,
                                        BITEXP_A, BITEXP_B,
                                        op0=A.mult, op1=A.add)

    for qb in range(QB):
        onat_t = work.tile([128, 4, 128], BF16, tag="onat", bufs=2, name="onat")
        onat_v = onat_t[:]
        for h in range(HPC):
            # full 2KB bank: a single psum accumulation group (zero region)
            # holds all four q-subtile accumulators; only the very first
            # matmul sets start, later subtiles self-zero via pending-zero.
            o_ps = psC.tile([128, 4, 128], F32, tag="o", name=f"ops{h}")
            opv = o_ps[:]
            prev = None
            # software pipeline: scores+exp for chunk n, then attn@V for n-1,
            # so the PE never sits behind an exp it doesn't depend on.
            for kc2 in range(KC // 2 + 1):
                cur = None
                if kc2 < KC // 2:
                    sc = psC.tile([128, 2, 512], F32, tag=f"sc{kc2 % 2}",
                                  name="sc")
                    scv = sc[:]
                    for i in range(2):
                        kc = kc2 * 2 + i
                        nc.tensor.matmul(
                            scv[:, i, :],
                            kT_v[64 * h:64 * h + 64, kc * 128:(kc + 1) * 128],
                            qT_v[64 * h:64 * h + 64, qb * 512:(qb + 1) * 512],
                            start=True, stop=True, tile_position=(64 * h, 0))
                    pt = work.tile([128, 2, 512], BF16, tag="pt", bufs=3,
                                   name="pt")
                    emit_exp(pt, scv)
                    if DEBUG and qb == 0 and h == 0 and kc2 == 0:
                        dump("d_pt00", pt[:])
                    cur = (pt, kc2)
                if prev is not None:
                    if pend_po:
                        emit_po(*pend_po.pop(0))
                    ptv, pc2 = prev[0][:], prev[1]
                    for i in range(2):
                        kc = pc2 * 2 + i
                        for qs in range(4):
                            nc.tensor.matmul(
                                opv[:, qs, 0:65],
                                ptv[:, i, qs * 128:(qs + 1) * 128],
                                vaug_v[:, kc, h, :],
                                start=(kc == 0 and qs == 0),
                                stop=(kc == KC - 1 and qs == 3),
                                skip_group_check=True)
                prev = cur
            if DEBUG and qb == 0 and h == 0:
                dump("d_ops00", opv[:, :, 0:65], psum=True)
            # normalize per q-partition: onat[:, qs, 64h:] = o[:, qs, 0:64]/den
            zr = work.tile([128, 4], F32, tag="zr", bufs=2)
            with nc.allow_low_precision(reason="recip"):
                nc.vector.reciprocal(zr[:], opv[:, :, 64])
            for qs in range(4):
                nc.vector.tensor_scalar(
                    onat_v[:, qs, 64 * h:64 * h + 64], opv[:, qs, 0:64],
                    zr[:, qs:qs + 1], None, op0=A.mult)
        # transpose o natural [q, d] -> oT [d, q]
        for qs2 in range(0, 4, 2):
            pb = psC.tile([128, 2, 128], BF16, tag="otr", bufs=1, name="otr")
            pbv = pb[:]
            for i in range(2):
                nc.tensor.transpose(pbv[:, i, :], onat_v[:, qs2 + i, :], identb[:])
            nc.vector.tensor_copy(
                oT_v[:, qb * 512 + qs2 * 128: qb * 512 + (qs2 + 2) * 128]
                .bitcast(U16),
                pbv.bitcast(U16))
        pend_po += [(qb, ts, half) for ts in range(4) for half in range(2)]
    if DEBUG:
        dump("d_oT", oT_v)
    for args in pend_po:
        emit_po(*args)
    psC.release()

    for p in (work, data, const):
        p.release()


# ---------------- host side ----------------

_PERM = np.concatenate([np.arange(0, HD, 2), np.arange(1, HD, 2)])
_TE, _TO = _PERM[:32], _PERM[32:]


def make_in_maps(x, y, pos, w_qkv_x, w_kv_y, w_proj, q_norm_w, k_norm_w,
                 n_tok, m_tok, ncores=NCORES):
    TT = n_tok // 128
    x2 = np.ascontiguousarray(x.reshape(n_tok, C).T).astype(BF)  # [C, n]
    y2 = np.ascontiguousarray(y.reshape(m_tok, C).T).astype(BF)
    cos = pos[:, :, 0].astype(np.float32)  # [n_tok, 32]
    sin = pos[:, :, 1].astype(np.float32)

    def coeff_tiles(w):
        we = w[_TE].astype(np.float32)
        wo = w[_TO].astype(np.float32)
        blocks = [cos * we, sin * wo, sin * we, cos * wo]  # cwe swo swe cwo
        arr = np.stack([b.reshape(TT, 128, 32).transpose(1, 0, 2) for b in blocks])
        return np.ascontiguousarray(
            arr.transpose(1, 0, 2, 3).reshape(128, 4 * TT * 32)).astype(BF)

    cq = coeff_tiles(q_norm_w)
    ck = coeff_tiles(k_norm_w)

    in_maps = []
    for cid in range(ncores):
        heads = [HPC * cid + i for i in range(HPC)]
        # channel order per head: [te(32), to(32)] (PERM); heads contiguous
        q_rows = np.concatenate([h * HD + _PERM for h in heads])
        kx_rows = C + q_rows
        vx_rows = 2 * C + np.concatenate([h * HD + np.arange(HD) for h in heads])
        wx = w_qkv_x[np.concatenate([q_rows, kx_rows, vx_rows])]  # [384, C]
        kyw = np.concatenate([k_norm_w[_PERM] for _ in heads])
        wyk = w_kv_y[q_rows] * kyw[:, None]     # fold k_norm into wy
        vy_rows = C + np.concatenate([h * HD + np.arange(HD) for h in heads])
        wy = np.concatenate([wyk, w_kv_y[vy_rows]])  # [256, C]
        wpc = w_proj[:, heads[0] * HD:(heads[-1] + 1) * HD].T  # [128 d, C]
        in_maps.append({
            "xT": x2, "yT": y2,
            "wx": np.ascontiguousarray(wx.T).astype(BF),
            "wy": np.ascontiguousarray(wy.T).astype(BF),
            "cq": cq, "ck": ck,
            "wp": np.ascontiguousarray(wpc).astype(BF),
        })
    return in_maps


_CACHE = {}


def _get_nc(n_tok, m_tok):
    key = (n_tok, m_tok)
    if key not in _CACHE:
        _CACHE[key] = build_nc(n_tok, m_tok)
    return _CACHE[key]


def run(x, y, pos, w_qkv_x, w_kv_y, w_proj, b_proj, q_norm_w, k_norm_w, **kw):
    B, n_tok, _ = x.shape
    m_tok = y.shape[1]
    nc = _get_nc(n_tok, m_tok)
    in_maps = make_in_maps(np.asarray(x), np.asarray(y), np.asarray(pos),
                           np.asarray(w_qkv_x), np.asarray(w_kv_y),
                           np.asarray(w_proj), np.asarray(q_norm_w),
                           np.asarray(k_norm_w), n_tok, m_tok)
    res = run_bass_kernel_spmd(nc, in_maps, core_ids=list(range(NCORES)), **kw)
    acc = np.zeros((n_tok, C), np.float32)
    for r in res.results:
        acc += r["out"].astype(np.float32)
    out = (acc + np.asarray(b_proj)[None, :].astype(np.float32)).astype(np.float32)
    return out.reshape(B, n_tok, C), res


def kernel(x, y, pos, w_qkv_x, w_kv_y, w_proj, b_proj, q_norm_w, k_norm_w):
    out, _ = run(x, y, pos, w_qkv_x, w_kv_y, w_proj, b_proj, q_norm_w, k_norm_w)
    return out
